# revision 1
# baseline (speedup 1.0000x reference)
"""Trainium2 Bass kernel for nn_AudioNetwork (4-block STFT resonator chain).

Algorithm notes
---------------
Per block: frame x (win 2048, hop 1024), rfft, per-bin linear recurrence over
frames out_i = (spec_i + out_{i-1}) * tc, irfft, hann-windowed overlap-add,
tanh(gain * s).  Since every recurrence step multiplies by tc, bins with
tc == 0 never contribute: the (i)DFT only needs the nonzero bins of tc
(~10 of 1025 for the reference init).  Both transforms become tiny matmuls.

Device layout (per core, 4 batch elements):
  x is kept "transposed": 8 SBUF tiles of (128 samples-in-chunk, 1024 cols)
  where col = batch*256 + frame-chunk index.  In this layout both the forward
  DFT (contract over the 1024 samples of a hop-chunk) and the inverse DFT
  (produce samples) are natural PE matmuls, so no transposes are needed
  inside the block chain — only once at load and once at store (PE-transpose
  via identity).

  Forward: spec_i needs frame i = [chunk_i, chunk_{i+1}] but
  cos/sin(2*pi*k*(s+1024)/2048) = (-1)^k * cos/sin(2*pi*k*s/2048), so only
  the half-window matrix U is computed; the second half is sign * U shifted
  by one frame.  The recurrence runs as a GpSimd tensor_tensor_scan per batch
  (state = state*tc + tc*spec).  Overlap-add is folded into the inverse
  matmul by stacking [outs; outs_shifted_one_frame] as the moving operand
  (shifted rows live at partition 64 so APs stay quadrant-aligned).

  DFT matmuls run in float32r (fast fp32 PE mode, ~1 col/cycle vs 4 for
  fp32; ~1e-4 matmul relative error, well inside tolerance).  Engine
  balance: PE does DFTs + layout transposes, ACT does tanh + half the
  transpose copies, DVE the other half + pointwise combines, GpSimd the
  scans and memsets.
"""

import numpy as np
from contextlib import ExitStack

import concourse.bass as bass
import concourse.tile as tile
from concourse import bacc, mybir, masks
from concourse import bass_utils

F32 = mybir.dt.float32
F32R = mybir.dt.float32r
WS = 2048
STEP = 1024
NCOEF = WS // 2 + 1
NBLK = 4
B = 32
T = 262144
NCORES = 8
BL = B // NCORES          # batch per core
NF = T // STEP            # 256 frames/chunks
KT = STEP // 128          # 8 K-tiles of the forward contraction
COLS = BL * NF            # 1024 free columns (batch-major)
MAX_BINS_PER_CHUNK = 32   # so shifted rows fit at partition 64

_CACHE = {}


def _plan_chunks(tc_vec):
    nz = np.nonzero(tc_vec)[0]
    if len(nz) == 0:
        nz = np.array([1], dtype=np.int64)  # dummy bin with tc=0: contributes 0
    chunks = [nz[i:i + MAX_BINS_PER_CHUNK] for i in range(0, len(nz), MAX_BINS_PER_CHUNK)]
    return chunks


def _host_matrices(tc_vec, chunks):
    """Build per-chunk constant arrays (float64 math, float32 storage)."""
    hann = 0.5 - 0.5 * np.cos(2.0 * np.pi * np.arange(WS) / WS)
    out = []
    for bins in chunks:
        nb = len(bins)
        k = bins.astype(np.float64)
        tcv = tc_vec[bins].astype(np.float64)
        s = np.arange(STEP, dtype=np.float64)
        ang = 2.0 * np.pi * np.outer(s, k) / WS                      # (1024, nb)
        # forward matrix padded to 64 rows: the matmul then writes exact
        # zeros into rows 2nb:64, so no memset is needed downstream
        bf = np.zeros((STEP, 64))
        bf[:, 0:nb] = np.cos(ang) * tcv
        bf[:, nb:2 * nb] = -np.sin(ang) * tcv
        bf_t = bf.reshape(KT, 128, 64).transpose(1, 0, 2)            # (128, 8, 64)
        sign = np.zeros((64, 1)); sign[0:nb, 0] = (-1.0) ** k; sign[nb:2 * nb, 0] = (-1.0) ** k
        tcrep = np.zeros((64, NF)); tcrep[0:nb] = tcv[:, None]; tcrep[nb:2 * nb] = tcv[:, None]
        w = np.where((bins == 0) | (bins == WS // 2), 1.0, 2.0)
        s2 = np.arange(WS, dtype=np.float64)
        ang2 = 2.0 * np.pi * np.outer(k, s2) / WS                    # (nb, 2048)
        are = (w[:, None] / WS) * np.cos(ang2) * hann
        aim = -(w[:, None] / WS) * np.sin(ang2) * hann
        w1 = np.concatenate([are[:, :STEP], aim[:, :STEP]], axis=0)  # (2nb, 1024) cur frame
        w2 = np.concatenate([are[:, STEP:], aim[:, STEP:]], axis=0)  # (2nb, 1024) prev frame
        # rows 2nb:64 (and 64+2nb:128) are zero: scat carries 64-row
        # current-frame and 64-row shifted operands
        pad = np.zeros((64 - 2 * nb, WS // 2))
        winv = np.concatenate([w1, pad, w2, pad], axis=0).reshape(128, KT, 128)
        out.append(dict(
            nb=nb,
            bf=np.ascontiguousarray(bf_t, dtype=np.float32),
            winv=np.ascontiguousarray(winv, dtype=np.float32),
            sign=np.ascontiguousarray(sign, dtype=np.float32),
            tcrep=np.ascontiguousarray(tcrep, dtype=np.float32),
        ))
    return out


def _build(chunk_sizes, gains, wmix):
    """Trace+compile the Bass program. chunk_sizes: tuple of tuples of nb per block."""
    nc = bacc.Bacc("TRN2", target_bir_lowering=False, debug=False)
    x_d = nc.dram_tensor("x", (BL, NF, STEP), F32, kind="ExternalInput").ap()
    out_d = nc.dram_tensor("out", (BL, NF, STEP), F32, kind="ExternalOutput").ap()
    cons = {}
    zc_d = nc.dram_tensor("zc", (64, BL, 1), F32, kind="ExternalInput").ap()
    for kb in range(NBLK):
        for c, nb in enumerate(chunk_sizes[kb]):
            cons[f"bf_{kb}_{c}"] = nc.dram_tensor(f"bf_{kb}_{c}", (128, KT, 64), F32, kind="ExternalInput").ap()
            cons[f"wi_{kb}_{c}"] = nc.dram_tensor(f"wi_{kb}_{c}", (128, KT, 128), F32, kind="ExternalInput").ap()
            cons[f"sg_{kb}_{c}"] = nc.dram_tensor(f"sg_{kb}_{c}", (64, 1), F32, kind="ExternalInput").ap()
            cons[f"tr_{kb}_{c}"] = nc.dram_tensor(f"tr_{kb}_{c}", (64, NF), F32, kind="ExternalInput").ap()

    mult = mybir.AluOpType.mult
    add = mybir.AluOpType.add

    dense = any(len(s) > 1 for s in chunk_sizes)
    nbufs = 2 if dense else 3
    with tile.TileContext(nc) as tc, ExitStack() as ctx:
        cpool = ctx.enter_context(tc.tile_pool(name="const", bufs=1))
        big = ctx.enter_context(tc.tile_pool(name="big", bufs=1))
        stream = ctx.enter_context(tc.tile_pool(name="stream", bufs=nbufs))
        wtp = ctx.enter_context(tc.tile_pool(name="wt", bufs=2))
        work = ctx.enter_context(tc.tile_pool(name="work", bufs=nbufs))
        # PSUM budget (8 banks): uv 1x2 + ips 2x2 + pst 2x1 = 8
        pmm = ctx.enter_context(tc.tile_pool(name="pmm", bufs=2, space="PSUM"))
        ptr = ctx.enter_context(tc.tile_pool(name="ptr", bufs=2, space="PSUM"))

        ident = cpool.tile([128, 128], F32)
        masks.make_identity(nc, ident[:])
        zc_t = cpool.tile([64, BL, 1], F32, name="zc_t")
        nc.sync.dma_start(zc_t[:], zc_d[:])

        # small per-chunk constants stay resident
        sg_t = {}
        tr_t = {}
        for kb in range(NBLK):
            for c, nb in enumerate(chunk_sizes[kb]):
                sg_t[(kb, c)] = cpool.tile([64, 1], F32, tag=f"sg{kb}_{c}", name=f"sg{kb}_{c}")
                nc.sync.dma_start(sg_t[(kb, c)][:], cons[f"sg_{kb}_{c}"][:])
                tr_t[(kb, c)] = cpool.tile([64, NF], F32, tag=f"tr{kb}_{c}", name=f"tr{kb}_{c}")
                nc.sync.dma_start(tr_t[(kb, c)][:], cons[f"tr_{kb}_{c}"][:])

        xbuf = [[big.tile([128, COLS], F32, tag=f"xb{i}_{a}", name=f"xb{i}_{a}") for a in range(KT)] for i in range(2)]
        accb = [big.tile([128, COLS], F32, tag=f"ac_{a}", name=f"ac_{a}") for a in range(KT)]

        def copy_engine(i):
            return nc.vector.tensor_copy if i % 2 == 0 else nc.scalar.copy

        # ---- load x and transpose into (sample, chunk) layout ----
        # two 128x128 transposes share one PSUM tile; copies split DVE/ACT
        for b in range(BL):
            for jt in range(2):
                xn = stream.tile([128, STEP], F32, tag="xnat")
                dma_eng = nc.sync if (b * 2 + jt) % 2 == 0 else nc.scalar
                dma_eng.dma_start(xn[:], x_d[b, jt * 128:(jt + 1) * 128, :])
                col = b * NF + jt * 128
                for ah in range(KT // 2):
                    pst = ptr.tile([128, 256], F32, tag="pst")
                    for i in range(2):
                        a = 2 * ah + i
                        nc.tensor.transpose(pst[:, i * 128:(i + 1) * 128],
                                            xn[:, a * 128:(a + 1) * 128], ident[:])
                    eng = copy_engine(ah)
                    eng(xbuf[0][2 * ah][:, col:col + 128], pst[:, 0:128])
                    eng(xbuf[0][2 * ah + 1][:, col:col + 128], pst[:, 128:256])
        # accumulator updates are deferred one block so they never compete
        # with the critical combine/scan chain on DVE
        pending_acc = [("init", a, xbuf[0][a], float(wmix[0])) for a in range(KT)]

        def flush_acc():
            for kind, m, t, w in pending_acc:
                if kind == "init":
                    nc.vector.tensor_scalar_mul(accb[m][:], t[:], w)
                else:
                    nc.vector.scalar_tensor_tensor(
                        accb[m][:], t[:], w, accb[m][:], op0=mult, op1=add)
            pending_acc.clear()

        # ---- block chain ----
        for kb in range(NBLK):
            src = xbuf[kb % 2]
            dst = xbuf[(kb + 1) % 2]
            sizes = chunk_sizes[kb]
            nch = len(sizes)
            inv_sb = None
            if nch > 1:
                inv_sb = [big.tile([128, COLS], F32, tag=f"is_{m}", name=f"is{kb}_{m}") for m in range(KT)]
            for c, nb in enumerate(sizes):
                bf = wtp.tile([128, KT, 64], F32, tag="bf")
                nc.sync.dma_start(bf[:], cons[f"bf_{kb}_{c}"][:])
                wi = wtp.tile([128, KT, 128], F32, tag="wi")
                nc.sync.dma_start(wi[:], cons[f"wi_{kb}_{c}"][:])

                uv = pmm.tile([64, BL, NF], F32, tag="uv", bufs=1)
                for g in range(2):
                    for a in range(KT):
                        nc.tensor.matmul(uv[:, 2 * g:2 * g + 2, :], bf[:, a, :],
                                         src[a][:, g * 512:(g + 1) * 512],
                                         start=(a == 0), stop=(a == KT - 1))
                # per-batch combine + scan + shift, so the inverse's first
                # column group unblocks after batches 0-1 instead of all four
                uvs = work.tile([64, BL, NF], F32, tag="uvs")
                in1 = work.tile([64, BL, NF], F32, tag="in1")
                scat = work.tile([128, BL, NF], F32, tag="scat")
                nc.sync.dma_start(scat[64:128, :, 0:1], zc_t[:])
                for b in range(BL):
                    nc.scalar.copy(uvs[:, b, :], uv[:, b, :])
                    nc.vector.scalar_tensor_tensor(
                        in1[:, b, 0:NF - 1], uvs[:, b, 1:NF], sg_t[(kb, c)][:, 0:1],
                        uv[:, b, 0:NF - 1], op0=mult, op1=add)
                    nc.vector.tensor_copy(in1[:, b, NF - 1:NF], uvs[:, b, NF - 1:NF])
                    nc.vector.tensor_tensor_scan(
                        scat[0:64, b, :], tr_t[(kb, c)][:], in1[:, b, :],
                        initial=0.0, op0=mult, op1=add)
                    nc.sync.dma_start(scat[64:128, b, 1:NF],
                                      scat[0:64, b, 0:NF - 1])
                flush_acc()
                # inverse DFT + hann + OLA
                for m in range(KT):
                    ps = pmm.tile([128, COLS], F32, tag="ips")
                    for g in range(2):
                        nc.tensor.matmul(ps[:, g * 512:(g + 1) * 512], wi[:, m, :],
                                         scat[:, 2 * g:2 * g + 2, :],
                                         start=True, stop=True)
                    if nch == 1:
                        nc.scalar.activation(dst[m][:], ps[:],
                                             mybir.ActivationFunctionType.Tanh,
                                             scale=float(gains[kb]))
                        pending_acc.append(("acc", m, dst[m], float(wmix[kb + 1])))
                    else:
                        if c == 0:
                            nc.vector.tensor_copy(inv_sb[m][:], ps[:])
                        else:
                            nc.vector.tensor_add(inv_sb[m][:], inv_sb[m][:], ps[:])
                        if c == nch - 1:
                            nc.scalar.activation(dst[m][:], inv_sb[m][:],
                                                 mybir.ActivationFunctionType.Tanh,
                                                 scale=float(gains[kb]))
                            pending_acc.append(("acc", m, dst[m], float(wmix[kb + 1])))

        flush_acc()

        # ---- transpose back and store ----
        for b in range(BL):
            for jt in range(2):
                on = stream.tile([128, STEP], F32, tag="onat")
                col = b * NF + jt * 128
                for ah in range(KT // 2):
                    pst = ptr.tile([128, 256], F32, tag="pst")
                    for i in range(2):
                        a = 2 * ah + i
                        nc.tensor.transpose(pst[:, i * 128:(i + 1) * 128],
                                            accb[a][:, col:col + 128], ident[:])
                    eng = copy_engine(ah + 1)
                    eng(on[:, (2 * ah) * 128:(2 * ah + 2) * 128], pst[:])
                dma_eng = nc.sync if (b * 2 + jt) % 2 == 0 else nc.scalar
                dma_eng.dma_start(out_d[b, jt * 128:(jt + 1) * 128, :], on[:])

    nc.compile()
    return nc


def kernel(x, transfers, gains, mixer):
    x = np.ascontiguousarray(np.asarray(x, dtype=np.float32))
    transfers = np.asarray(transfers, dtype=np.float32)
    gains = np.asarray(gains, dtype=np.float64)
    mixer = np.asarray(mixer, dtype=np.float64)
    wm = np.exp(mixer - mixer.max())
    wm = wm / wm.sum()

    plans = [_plan_chunks(transfers[kb]) for kb in range(NBLK)]
    chunk_sizes = tuple(tuple(len(ch) for ch in pl) for pl in plans)
    key = (chunk_sizes, tuple(np.round(gains, 9)), tuple(np.round(wm, 9)))
    if key not in _CACHE:
        _CACHE[key] = _build(chunk_sizes, gains, wm)
    nc = _CACHE[key]

    const_map = {"zc": np.zeros((64, BL, 1), dtype=np.float32)}
    for kb in range(NBLK):
        mats = _host_matrices(transfers[kb].astype(np.float64), plans[kb])
        for c, md in enumerate(mats):
            const_map[f"bf_{kb}_{c}"] = md["bf"]
            const_map[f"wi_{kb}_{c}"] = md["winv"]
            const_map[f"sg_{kb}_{c}"] = md["sign"]
            const_map[f"tr_{kb}_{c}"] = md["tcrep"]

    xr = x.reshape(B, T)
    in_maps = []
    for core in range(NCORES):
        m = dict(const_map)
        m["x"] = np.ascontiguousarray(xr[core * BL:(core + 1) * BL].reshape(BL, NF, STEP))
        in_maps.append(m)

    res = bass_utils.run_bass_kernel_spmd(nc, in_maps, core_ids=list(range(NCORES)))
    out = np.concatenate([res.results[i]["out"].reshape(BL, 1, T) for i in range(NCORES)], axis=0)
    return out.astype(np.float32)



# revision 17
# speedup vs baseline: 1.3212x; 1.3212x over previous
"""Trainium2 Bass kernel for nn_AudioNetwork (4-block STFT resonator chain).

Algorithm notes
---------------
Per block: frame x (win 2048, hop 1024), rfft, per-bin linear recurrence over
frames out_i = (spec_i + out_{i-1}) * tc, irfft, hann-windowed overlap-add,
tanh(gain * s).  Since every recurrence step multiplies by tc, bins with
tc == 0 never contribute: the (i)DFT only needs the nonzero bins of tc
(~10 of 1025 for the reference init).  Both transforms become tiny matmuls.

Device layout (per core, 4 batch elements), v2 (fp16 pipeline):
  The signal lives in SBUF as fp16 tiles (128 samples-in-chunk, KT, 1024 cols)
  where col = batch*256 + chunk.  Layout changes use the DMA crossbar
  transpose (dma_start_transpose, 16x128 xbar tiles) instead of PE
  transposes, so the PE only runs the DFT matmuls.  HBM<->SBUF transfers use
  gpsimd (software DGE) DMAs, which cast fp32<->fp16 in flight.

  Forward: spec_i needs frame i = [chunk_i, chunk_{i+1}] but
  cos/sin(2*pi*k*(s+1024)/2048) = (-1)^k * cos/sin(2*pi*k*s/2048), so only
  the half-window matrix U is computed; the second half is sign * U shifted
  by one frame.  The forward stationary is duplicated [bf|bf] so U lands on
  both partition halves (same PE cost - cycles scale with moving cols only).
  Two partition-aligned scans per batch (rows 0:64 -> cols 1.., rows 64:128
  -> cols 2..) then give the [current; one-frame-delayed] stack the inverse
  needs with a single uniform column offset, with no cross-partition copies
  or DMAs.  The scan state is fp32 internally regardless of operand dtype,
  so the fp16 recurrence does not accumulate rounding.  tr stays fp32: a
  2^-11 error in tc compounds over 256 frames.

  The inverse DFT matrices are scaled by 2**10 (compensated in the tanh
  scale) so the hann-window tails stay in fp16 normal range.  The mixer
  accumulator is fp16 and updated with scalar_tensor_tensor one block late
  so it never competes with the combine/scan chain on DVE.
"""

import numpy as np
from contextlib import ExitStack

import concourse.bass as bass
import concourse.tile as tile
from concourse import bacc, mybir
from concourse import bass_utils

F32 = mybir.dt.float32
F16 = mybir.dt.float16
WS = 2048
STEP = 1024
NCOEF = WS // 2 + 1
NBLK = 4
B = 32
T = 262144
NCORES = 8
BL = B // NCORES          # batch per core
NF = T // STEP            # 256 frames/chunks
KT = STEP // 128          # 8 K-tiles of the forward contraction
COLS = BL * NF            # 1024 free columns (batch-major)
MAX_BINS_PER_CHUNK = 32   # 2*nb must fit in a 64-row half
WI_SCALE = 1024.0         # keeps hann tails in fp16 normal range

_CACHE = {}


def _plan_chunks(tc_vec):
    nz = np.nonzero(tc_vec)[0]
    if len(nz) == 0:
        nz = np.array([1], dtype=np.int64)  # dummy bin with tc=0: contributes 0
    chunks = [nz[i:i + MAX_BINS_PER_CHUNK] for i in range(0, len(nz), MAX_BINS_PER_CHUNK)]
    return chunks


def _host_matrices(tc_vec, chunks):
    """Build per-chunk constant arrays (float64 math, fp16/fp32 storage)."""
    hann = 0.5 - 0.5 * np.cos(2.0 * np.pi * np.arange(WS) / WS)
    out = []
    for bins in chunks:
        nb = len(bins)
        k = bins.astype(np.float64)
        tcv = tc_vec[bins].astype(np.float64)
        s = np.arange(STEP, dtype=np.float64)
        ang = 2.0 * np.pi * np.outer(s, k) / WS                      # (1024, nb)
        # forward matrix, duplicated on both column halves: the matmul then
        # writes U to partitions 0:64 and 64:128 at no extra PE cost
        bf = np.zeros((STEP, 128))
        bf[:, 0:nb] = np.cos(ang) * tcv
        bf[:, nb:2 * nb] = -np.sin(ang) * tcv
        bf[:, 64:64 + 2 * nb] = bf[:, 0:2 * nb]
        bf_t = bf.reshape(KT, 128, 128).transpose(1, 0, 2)           # (128, 8, 128)
        sign = np.zeros((128, 1))
        sign[0:nb, 0] = (-1.0) ** k
        sign[nb:2 * nb, 0] = (-1.0) ** k
        sign[64:64 + 2 * nb] = sign[0:2 * nb]
        tcrep = np.zeros((128, NF))
        tcrep[0:nb] = tcv[:, None]
        tcrep[nb:2 * nb] = tcv[:, None]
        tcrep[64:64 + 2 * nb] = tcrep[0:2 * nb]
        w = np.where((bins == 0) | (bins == WS // 2), 1.0, 2.0)
        s2 = np.arange(WS, dtype=np.float64)
        ang2 = 2.0 * np.pi * np.outer(k, s2) / WS                    # (nb, 2048)
        are = (w[:, None] / WS) * np.cos(ang2) * hann * WI_SCALE
        aim = -(w[:, None] / WS) * np.sin(ang2) * hann * WI_SCALE
        w1 = np.concatenate([are[:, :STEP], aim[:, :STEP]], axis=0)  # cur frame
        w2 = np.concatenate([are[:, STEP:], aim[:, STEP:]], axis=0)  # prev frame
        pad = np.zeros((64 - 2 * nb, WS // 2))
        winv = np.concatenate([w1, pad, w2, pad], axis=0).reshape(128, KT, 128)
        out.append(dict(
            nb=nb,
            bf=np.ascontiguousarray(bf_t, dtype=np.float16),
            winv=np.ascontiguousarray(winv, dtype=np.float16),
            sign=np.ascontiguousarray(sign, dtype=np.float16),
            tcrep=np.ascontiguousarray(tcrep, dtype=np.float32),
        ))
    return out


def _build(chunk_sizes, gains, wmix, debug_taps=False):
    """Trace+compile the Bass program. chunk_sizes: tuple of tuples of nb per block."""
    nc = bacc.Bacc("TRN2", target_bir_lowering=False, debug=False)
    x_d = nc.dram_tensor("x", (BL, NF, STEP), F32, kind="ExternalInput").ap()
    out_d = nc.dram_tensor("out", (BL, NF, STEP), F32, kind="ExternalOutput").ap()
    taps = {}
    if debug_taps:
        for nm, shp in [("t_x16", (128, KT, COLS)), ("t_dst1", (128, KT, COLS)),
                        ("t_scat1", (128, BL, NF + 1)), ("t_uvs1", (128, BL, NF)),
                        ("t_acc", (128, KT, COLS)), ("t_onat", (128, 2 * BL, KT, 128))]:
            taps[nm] = nc.dram_tensor(nm, shp, F16, kind="ExternalOutput").ap()
    cons = {}
    for kb in range(NBLK):
        for c, nb in enumerate(chunk_sizes[kb]):
            cons[f"bf_{kb}_{c}"] = nc.dram_tensor(f"bf_{kb}_{c}", (128, KT, 128), F16, kind="ExternalInput").ap()
            cons[f"wi_{kb}_{c}"] = nc.dram_tensor(f"wi_{kb}_{c}", (128, KT, 128), F16, kind="ExternalInput").ap()
            cons[f"sg_{kb}_{c}"] = nc.dram_tensor(f"sg_{kb}_{c}", (128, 1), F16, kind="ExternalInput").ap()
            cons[f"tr_{kb}_{c}"] = nc.dram_tensor(f"tr_{kb}_{c}", (128, NF), F32, kind="ExternalInput").ap()

    mult = mybir.AluOpType.mult
    add = mybir.AluOpType.add
    Tanh = mybir.ActivationFunctionType.Tanh

    dense = any(len(s) > 1 for s in chunk_sizes)
    with tile.TileContext(nc) as tc, ExitStack() as ctx:
        cpool = ctx.enter_context(tc.tile_pool(name="const", bufs=1))
        big = ctx.enter_context(tc.tile_pool(name="big", bufs=1))
        stream = ctx.enter_context(tc.tile_pool(name="stream", bufs=3))
        wtp = ctx.enter_context(tc.tile_pool(name="wt", bufs=2))
        work = ctx.enter_context(tc.tile_pool(name="work", bufs=2))
        # PSUM budget (8 banks): uv 2x2 + ips 2x2 = 8
        pmm = ctx.enter_context(tc.tile_pool(name="pmm", bufs=2, space="PSUM"))

        # small per-chunk constants stay resident
        sg_t = {}
        tr_t = {}
        for kb in range(NBLK):
            for c, nb in enumerate(chunk_sizes[kb]):
                sg_t[(kb, c)] = cpool.tile([128, 1], F16, tag=f"sg{kb}_{c}", name=f"sg{kb}_{c}")
                nc.sync.dma_start(sg_t[(kb, c)][:], cons[f"sg_{kb}_{c}"][:])
                tr_t[(kb, c)] = cpool.tile([128, NF], F32, tag=f"tr{kb}_{c}", name=f"tr{kb}_{c}")
                nc.sync.dma_start(tr_t[(kb, c)][:], cons[f"tr_{kb}_{c}"][:])

        xbuf = [big.tile([128, KT, COLS], F16, tag=f"xb{i}", name=f"xb{i}") for i in range(2)]
        accb = big.tile([128, KT, COLS], F16, tag="acc", name="acc")
        onat = big.tile([128, 2 * BL, KT, 128], F16, tag="onat", name="onat")
        # scat col k: rows 0:64 = out_{k-1}, rows 64:128 = out_{k-2}; col 0
        # (and col 1 of the delayed half) stay zero for the overlap-add edge
        scat = cpool.tile([128, BL, NF + 1], F16, tag="scat", name="scat")
        nc.vector.memset(scat[:, :, 0:2], 0.0)

        # ---- load x: DMA fp32, cast to fp16, then crossbar-transpose ----
        for b in range(BL):
            for jt in range(2):
                xn32 = stream.tile([128, STEP], F32, tag="xn32")
                dma_eng = nc.sync if (b * 2 + jt) % 2 == 0 else nc.scalar
                dma_eng.dma_start(xn32[:], x_d[b, jt * 128:(jt + 1) * 128, :])
                xn = stream.tile([128, STEP], F16, tag="xn")
                cast_eng = nc.vector.tensor_copy if (b * 2 + jt) % 2 == 0 else nc.scalar.copy
                cast_eng(xn[:], xn32[:])
                col = b * NF + jt * 128
                # NOTE: all crossbar transposes ride one queue — concurrent
                # xbar DMAs from two queues clobber adjacent destination
                # slices (observed nondeterministic corruption on HW)
                nc.sync.dma_start_transpose(xbuf[0][:, :, col:col + 128], xn[:])
                # mixer accumulator init (w0 * x), off the critical path
                nc.vector.tensor_scalar_mul(
                    accb[:, :, col:col + 128], xbuf[0][:, :, col:col + 128],
                    float(wmix[0]))

        if debug_taps:
            nc.sync.dma_start(taps["t_x16"][:], xbuf[0][:])

        pending_acc = []

        def flush_acc():
            for m, t, w in pending_acc:
                nc.vector.scalar_tensor_tensor(
                    accb[:, m, :], t[:, m, :], w, accb[:, m, :], op0=mult, op1=add)
            pending_acc.clear()

        # ---- block chain ----
        for kb in range(NBLK):
            if debug_taps and kb == 1:
                nc.sync.dma_start(taps["t_dst1"][:], xbuf[1][:])
            src = xbuf[kb % 2]
            dst = xbuf[(kb + 1) % 2]
            sizes = chunk_sizes[kb]
            nch = len(sizes)
            inv_sb = None
            if nch > 1:
                inv_sb = big.tile([128, KT, COLS], F32, tag="is", name=f"is{kb}")
            for c, nb in enumerate(sizes):
                bf = wtp.tile([128, KT, 128], F16, tag="bf")
                nc.sync.dma_start(bf[:], cons[f"bf_{kb}_{c}"][:])
                wi = wtp.tile([128, KT, 128], F16, tag="wi")
                nc.sync.dma_start(wi[:], cons[f"wi_{kb}_{c}"][:])

                uv = pmm.tile([128, BL, NF], F32, tag="uv")
                for a in range(KT):
                    for g in range(2):
                        nc.tensor.matmul(uv[:, 2 * g:2 * g + 2, :], bf[:, a, :],
                                         src[:, a, g * 512:(g + 1) * 512],
                                         start=(a == 0), stop=(a == KT - 1))
                # per-batch combine + two shifted scans (state fp32 internally)
                uvs = work.tile([128, BL, NF], F16, tag="uvs")
                in1 = work.tile([128, BL, NF], F16, tag="in1")
                for b in range(BL):
                    nc.scalar.copy(uvs[:, b, :], uv[:, b, :])
                    nc.vector.scalar_tensor_tensor(
                        in1[:, b, 0:NF - 1], uvs[:, b, 1:NF], sg_t[(kb, c)][:, 0:1],
                        uvs[:, b, 0:NF - 1], op0=mult, op1=add)
                    nc.vector.tensor_copy(in1[:, b, NF - 1:NF], uvs[:, b, NF - 1:NF])
                    nc.vector.tensor_tensor_scan(
                        scat[0:64, b, 1:NF + 1], tr_t[(kb, c)][0:64, :], in1[0:64, b, :],
                        initial=0.0, op0=mult, op1=add)
                    nc.vector.tensor_tensor_scan(
                        scat[64:128, b, 2:NF + 1], tr_t[(kb, c)][64:128, 0:NF - 1],
                        in1[64:128, b, 0:NF - 1],
                        initial=0.0, op0=mult, op1=add)
                if debug_taps and kb == 0:
                    nc.scalar.dma_start(taps["t_scat1"][:], scat[:])
                    nc.scalar.dma_start(taps["t_uvs1"][:], uvs[:])
                flush_acc()
                # inverse DFT + hann + OLA (overlap fold via the col-0 zeros)
                for m in range(KT):
                    ips = pmm.tile([128, COLS], F32, tag="ips")
                    for g in range(2):
                        nc.tensor.matmul(ips[:, g * 512:(g + 1) * 512], wi[:, m, :],
                                         scat[:, 2 * g:2 * g + 2, 1:NF + 1],
                                         start=True, stop=True)
                    if nch == 1:
                        nc.scalar.activation(dst[:, m, :], ips[:], Tanh,
                                             scale=float(gains[kb]) / WI_SCALE)
                        pending_acc.append((m, dst, float(wmix[kb + 1])))
                    else:
                        if c == 0:
                            nc.vector.tensor_copy(inv_sb[:, m, :], ips[:])
                        else:
                            nc.vector.tensor_add(inv_sb[:, m, :], inv_sb[:, m, :], ips[:])
                        if c == nch - 1:
                            nc.scalar.activation(dst[:, m, :], inv_sb[:, m, :], Tanh,
                                                 scale=float(gains[kb]) / WI_SCALE)
                            pending_acc.append((m, dst, float(wmix[kb + 1])))

        flush_acc()
        if debug_taps:
            nc.sync.dma_start(taps["t_acc"][:], accb[:])

        # ---- crossbar-transpose back, cast to fp32, and store ----
        for m in range(KT):
            nc.sync.dma_start_transpose(onat[:, :, m, :], accb[:, m, :])
        if debug_taps:
            nc.sync.dma_start(taps["t_onat"][:], onat[:])
        for b in range(BL):
            for jt in range(2):
                on32 = stream.tile([128, STEP], F32, tag="on32")
                cast_eng = nc.vector.tensor_copy if (b * 2 + jt) % 2 == 0 else nc.scalar.copy
                cast_eng(on32[:], onat[:, b * 2 + jt, :, :])
                dma_eng = nc.sync if (b * 2 + jt) % 2 == 0 else nc.scalar
                dma_eng.dma_start(out_d[b, jt * 128:(jt + 1) * 128, :], on32[:])

    nc.compile()
    return nc


def _const_map(transfers, plans):
    cm = {}
    for kb in range(NBLK):
        mats = _host_matrices(transfers[kb].astype(np.float64), plans[kb])
        for c, md in enumerate(mats):
            cm[f"bf_{kb}_{c}"] = md["bf"]
            cm[f"wi_{kb}_{c}"] = md["winv"]
            cm[f"sg_{kb}_{c}"] = md["sign"]
            cm[f"tr_{kb}_{c}"] = md["tcrep"]
    return cm


def _in_maps(x, const_map):
    xr = np.ascontiguousarray(np.asarray(x, dtype=np.float32)).reshape(B, T)
    maps = []
    for core in range(NCORES):
        m = dict(const_map)
        m["x"] = np.ascontiguousarray(xr[core * BL:(core + 1) * BL].reshape(BL, NF, STEP))
        maps.append(m)
    return maps


def kernel(x, transfers, gains, mixer):
    transfers = np.asarray(transfers, dtype=np.float32)
    gains = np.asarray(gains, dtype=np.float64)
    mixer = np.asarray(mixer, dtype=np.float64)
    wm = np.exp(mixer - mixer.max())
    wm = wm / wm.sum()

    plans = [_plan_chunks(transfers[kb]) for kb in range(NBLK)]
    chunk_sizes = tuple(tuple(len(ch) for ch in pl) for pl in plans)
    key = (chunk_sizes, tuple(np.round(gains, 9)), tuple(np.round(wm, 9)))
    if key not in _CACHE:
        _CACHE[key] = _build(chunk_sizes, gains, wm)
    nc = _CACHE[key]

    in_maps = _in_maps(x, _const_map(transfers, plans))
    res = bass_utils.run_bass_kernel_spmd(nc, in_maps, core_ids=list(range(NCORES)))
    out = np.concatenate([res.results[i]["out"].reshape(BL, 1, T) for i in range(NCORES)], axis=0)
    return out.astype(np.float32)


# revision 20
# speedup vs baseline: 1.3794x; 1.0441x over previous
"""Trainium2 Bass kernel for nn_AudioNetwork (4-block STFT resonator chain).

Algorithm notes
---------------
Per block: frame x (win 2048, hop 1024), rfft, per-bin linear recurrence over
frames out_i = (spec_i + out_{i-1}) * tc, irfft, hann-windowed overlap-add,
tanh(gain * s).  Since every recurrence step multiplies by tc, bins with
tc == 0 never contribute: the (i)DFT only needs the nonzero bins of tc
(~10 of 1025 for the reference init).  Both transforms become tiny matmuls.

Device layout (per core, 4 batch elements), v3 (fp16 pipeline):
  The signal lives in SBUF as fp16 tiles (128 samples-in-chunk, KT, 1024 cols)
  where col = batch*256 + chunk.  Layout changes use the DMA crossbar
  transpose (dma_start_transpose, 16x128 xbar tiles) instead of PE
  transposes, so the PE only runs the DFT matmuls.  All crossbar transposes
  ride ONE queue: concurrent xbar DMAs from two queues clobber adjacent
  destination slices (observed as nondeterministic corruption on HW).
  HBM->SBUF input loads are gpsimd (software DGE) DMAs casting fp32->fp16
  in flight.

  Forward: spec_i needs frame i = [chunk_i, chunk_{i+1}] but
  cos/sin(2*pi*k*(s+1024)/2048) = (-1)^k * cos/sin(2*pi*k*s/2048), so only
  the half-window matrix U is computed; the second half is sign * U shifted
  by one frame.  The forward runs column-group-first (batches 0-1 then 2-3)
  so the recurrence work starts while the PE is still on batches 2-3.  The
  scan state is fp32 internally regardless of operand dtype, so the fp16
  recurrence does not accumulate rounding; tr stays fp32 since a 2^-11
  error in tc compounds over 256 frames.  The inverse needs the stack
  [out_cur; out_prev]: the scan writes rows 0:64 of scat at col k+1, and a
  GpSimd cross-partition copy fills rows 64:128 one column later (the
  leading memset zero provides the overlap-add edge), giving the inverse a
  single 128-row stationary with one uniform column offset.

  The inverse DFT matrices are scaled by 2**10 (compensated in the tanh
  scale) so the hann-window tails stay in fp16 normal range.  When the
  mixer weights are all equal (softmax of the zero mixer), the accumulator
  is plain adds (split DVE/GpSimd) with the weight folded into the store
  cast; accumulator updates run one block late so they never compete with
  the combine/scan chain.
"""

import numpy as np
from contextlib import ExitStack

import concourse.bass as bass
import concourse.tile as tile
from concourse import bacc, mybir
from concourse import bass_utils

F32 = mybir.dt.float32
F16 = mybir.dt.float16
WS = 2048
STEP = 1024
NCOEF = WS // 2 + 1
NBLK = 4
B = 32
T = 262144
NCORES = 8
BL = B // NCORES          # batch per core
NF = T // STEP            # 256 frames/chunks
KT = STEP // 128          # 8 K-tiles of the forward contraction
COLS = BL * NF            # 1024 free columns (batch-major)
MAX_BINS_PER_CHUNK = 32   # 2*nb must fit in a 64-row half
WI_SCALE = 1024.0         # keeps hann tails in fp16 normal range

_CACHE = {}


def _plan_chunks(tc_vec):
    nz = np.nonzero(tc_vec)[0]
    if len(nz) == 0:
        nz = np.array([1], dtype=np.int64)  # dummy bin with tc=0: contributes 0
    chunks = [nz[i:i + MAX_BINS_PER_CHUNK] for i in range(0, len(nz), MAX_BINS_PER_CHUNK)]
    return chunks


def _host_matrices(tc_vec, chunks):
    """Build per-chunk constant arrays (float64 math, fp16/fp32 storage)."""
    hann = 0.5 - 0.5 * np.cos(2.0 * np.pi * np.arange(WS) / WS)
    out = []
    for bins in chunks:
        nb = len(bins)
        k = bins.astype(np.float64)
        tcv = tc_vec[bins].astype(np.float64)
        s = np.arange(STEP, dtype=np.float64)
        ang = 2.0 * np.pi * np.outer(s, k) / WS                      # (1024, nb)
        bf = np.zeros((STEP, 64))
        bf[:, 0:nb] = np.cos(ang) * tcv
        bf[:, nb:2 * nb] = -np.sin(ang) * tcv
        bf_t = bf.reshape(KT, 128, 64).transpose(1, 0, 2)            # (128, 8, 64)
        sign = np.zeros((64, 1))
        sign[0:nb, 0] = (-1.0) ** k
        sign[nb:2 * nb, 0] = (-1.0) ** k
        tcrep = np.zeros((64, NF))
        tcrep[0:nb] = tcv[:, None]
        tcrep[nb:2 * nb] = tcv[:, None]
        w = np.where((bins == 0) | (bins == WS // 2), 1.0, 2.0)
        s2 = np.arange(WS, dtype=np.float64)
        ang2 = 2.0 * np.pi * np.outer(k, s2) / WS                    # (nb, 2048)
        are = (w[:, None] / WS) * np.cos(ang2) * hann * WI_SCALE
        aim = -(w[:, None] / WS) * np.sin(ang2) * hann * WI_SCALE
        w1 = np.concatenate([are[:, :STEP], aim[:, :STEP]], axis=0)  # cur frame
        w2 = np.concatenate([are[:, STEP:], aim[:, STEP:]], axis=0)  # prev frame
        pad = np.zeros((64 - 2 * nb, WS // 2))
        winv = np.concatenate([w1, pad, w2, pad], axis=0).reshape(128, KT, 128)
        out.append(dict(
            nb=nb,
            bf=np.ascontiguousarray(bf_t, dtype=np.float16),
            winv=np.ascontiguousarray(winv, dtype=np.float16),
            sign=np.ascontiguousarray(sign, dtype=np.float16),
            tcrep=np.ascontiguousarray(tcrep, dtype=np.float32),
        ))
    return out


def _build(chunk_sizes, gains, wmix, debug_taps=False):
    """Trace+compile the Bass program. chunk_sizes: tuple of tuples of nb per block."""
    nc = bacc.Bacc("TRN2", target_bir_lowering=False, debug=False)
    x_d = nc.dram_tensor("x", (BL, NF, STEP), F32, kind="ExternalInput").ap()
    out_d = nc.dram_tensor("out", (BL, NF, STEP), F32, kind="ExternalOutput").ap()
    taps = {}
    if debug_taps:
        for nm, shp in [("t_x16", (128, KT, COLS)), ("t_dst1", (128, KT, COLS)),
                        ("t_scat1", (128, BL, NF + 1)), ("t_acc", (128, KT, COLS)),
                        ("t_onat", (128, 2 * BL, KT, 128))]:
            taps[nm] = nc.dram_tensor(nm, shp, F16, kind="ExternalOutput").ap()
    cons = {}
    for kb in range(NBLK):
        for c, nb in enumerate(chunk_sizes[kb]):
            cons[f"bf_{kb}_{c}"] = nc.dram_tensor(f"bf_{kb}_{c}", (128, KT, 64), F16, kind="ExternalInput").ap()
            cons[f"wi_{kb}_{c}"] = nc.dram_tensor(f"wi_{kb}_{c}", (128, KT, 128), F16, kind="ExternalInput").ap()
            cons[f"sg_{kb}_{c}"] = nc.dram_tensor(f"sg_{kb}_{c}", (64, 1), F16, kind="ExternalInput").ap()
            cons[f"tr_{kb}_{c}"] = nc.dram_tensor(f"tr_{kb}_{c}", (64, NF), F32, kind="ExternalInput").ap()

    mult = mybir.AluOpType.mult
    add = mybir.AluOpType.add
    Tanh = mybir.ActivationFunctionType.Tanh
    Copy = mybir.ActivationFunctionType.Copy

    w_equal = bool(np.allclose(wmix, wmix[0], rtol=1e-7, atol=0.0))

    with tile.TileContext(nc) as tc, ExitStack() as ctx:
        cpool = ctx.enter_context(tc.tile_pool(name="const", bufs=1))
        big = ctx.enter_context(tc.tile_pool(name="big", bufs=1))
        stream = ctx.enter_context(tc.tile_pool(name="stream", bufs=3))
        wtp = ctx.enter_context(tc.tile_pool(name="wt", bufs=2))
        work = ctx.enter_context(tc.tile_pool(name="work", bufs=2))
        # PSUM budget (8 banks): uv 2x2 + ips 2x2 = 8
        pmm = ctx.enter_context(tc.tile_pool(name="pmm", bufs=2, space="PSUM"))

        # small per-chunk constants stay resident
        sg_t = {}
        tr_t = {}
        for kb in range(NBLK):
            for c, nb in enumerate(chunk_sizes[kb]):
                sg_t[(kb, c)] = cpool.tile([64, 1], F16, tag=f"sg{kb}_{c}", name=f"sg{kb}_{c}")
                nc.sync.dma_start(sg_t[(kb, c)][:], cons[f"sg_{kb}_{c}"][:])
                tr_t[(kb, c)] = cpool.tile([64, NF], F32, tag=f"tr{kb}_{c}", name=f"tr{kb}_{c}")
                nc.sync.dma_start(tr_t[(kb, c)][:], cons[f"tr_{kb}_{c}"][:])

        xbuf = [big.tile([128, KT, COLS], F16, tag=f"xb{i}", name=f"xb{i}") for i in range(2)]
        accb = big.tile([128, KT, COLS], F16, tag="acc", name="acc")
        onat = big.tile([128, 2 * BL, KT, 128], F16, tag="onat", name="onat")
        # scat col k: rows 0:64 = out_{k-1} (scan), rows 64:128 = out_{k-2}
        # (cross-partition shifted copy); col 0 zero feeds the overlap edge
        scat = cpool.tile([128, BL, NF + 1], F16, tag="scat", name="scat")
        nc.vector.memset(scat[:, :, 0:1], 0.0)
        # uvs col NF stays zero: the sign-combine then covers all 256 cols
        uvs = cpool.tile([64, BL, NF + 1], F16, tag="uvs", name="uvs")
        nc.vector.memset(uvs[:, :, NF:NF + 1], 0.0)

        # ---- load x (casting DMA) + crossbar-transpose + accumulator init ----
        for b in range(BL):
            for jt in range(2):
                xn = stream.tile([128, STEP], F16, tag="xn")
                nc.gpsimd.dma_start(xn[:], x_d[b, jt * 128:(jt + 1) * 128, :])
                col = b * NF + jt * 128
                nc.sync.dma_start_transpose(xbuf[0][:, :, col:col + 128], xn[:])
                if w_equal:
                    eng = nc.scalar.copy if (b * 2 + jt) % 2 == 0 else nc.vector.tensor_copy
                    eng(accb[:, :, col:col + 128], xbuf[0][:, :, col:col + 128])
                else:
                    nc.vector.tensor_scalar_mul(
                        accb[:, :, col:col + 128], xbuf[0][:, :, col:col + 128],
                        float(wmix[0]))

        if debug_taps:
            nc.sync.dma_start(taps["t_x16"][:], xbuf[0][:])

        pending_acc = []

        def flush_acc(interleave=None):
            for i, (m, t, w) in enumerate(pending_acc):
                if w_equal:
                    eng = nc.gpsimd if i >= 5 else nc.vector
                    eng.tensor_tensor(accb[:, m, :], accb[:, m, :], t[:, m, :], op=add)
                else:
                    nc.vector.scalar_tensor_tensor(
                        accb[:, m, :], t[:, m, :], w, accb[:, m, :], op0=mult, op1=add)
                if interleave is not None:
                    interleave(m)
            pending_acc.clear()

        # ---- block chain ----
        for kb in range(NBLK):
            if debug_taps and kb == 1:
                nc.sync.dma_start(taps["t_dst1"][:], xbuf[1][:])
            src = xbuf[kb % 2]
            dst = xbuf[(kb + 1) % 2]
            sizes = chunk_sizes[kb]
            nch = len(sizes)
            inv_sb = None
            if nch > 1:
                inv_sb = big.tile([128, KT, COLS], F32, tag="is", name=f"is{kb}")
            for c, nb in enumerate(sizes):
                bf = wtp.tile([128, KT, 64], F16, tag="bf")
                nc.sync.dma_start(bf[:], cons[f"bf_{kb}_{c}"][:])
                wi = wtp.tile([128, KT, 128], F16, tag="wi")
                nc.sync.dma_start(wi[:], cons[f"wi_{kb}_{c}"][:])

                # forward, column-group-first: batches 0-1 finish early so the
                # recurrence overlaps the second half's matmuls
                uv = pmm.tile([64, BL, NF], F32, tag="uv")
                for g in range(2):
                    for a in range(KT):
                        nc.tensor.matmul(uv[:, 2 * g:2 * g + 2, :], bf[:, a, :],
                                         src[:, a, g * 512:(g + 1) * 512],
                                         start=(a == 0), stop=(a == KT - 1))
                    for b in (2 * g, 2 * g + 1):
                        nc.scalar.copy(uvs[:, b, 0:NF], uv[:, b, :])
                        in1 = work.tile([64, NF], F16, tag="in1")
                        nc.vector.scalar_tensor_tensor(
                            in1[:], uvs[:, b, 1:NF + 1], sg_t[(kb, c)][:, 0:1],
                            uvs[:, b, 0:NF], op0=mult, op1=add)
                        nc.vector.tensor_tensor_scan(
                            scat[0:64, b, 1:NF + 1], tr_t[(kb, c)][:], in1[:],
                            initial=0.0, op0=mult, op1=add)
                        nc.gpsimd.tensor_copy(scat[64:128, b, 1:NF + 1],
                                              scat[0:64, b, 0:NF])
                if debug_taps and kb == 0:
                    nc.scalar.dma_start(taps["t_scat1"][:], scat[:])
                flush_acc()
                # inverse DFT + hann + OLA, pair-staggered so the first
                # column group proceeds while batches 2-3 are still scanning
                ips = {}
                for mp in range(KT // 2):
                    m0, m1 = 2 * mp, 2 * mp + 1
                    ips[m0] = pmm.tile([128, COLS], F32, tag="ips", name=f"ips{m0}")
                    ips[m1] = pmm.tile([128, COLS], F32, tag="ips", name=f"ips{m1}")
                    for g, m in ((0, m0), (0, m1), (1, m0), (1, m1)):
                        nc.tensor.matmul(
                            ips[m][:, g * 512:(g + 1) * 512], wi[:, m, :],
                            scat[:, 2 * g:2 * g + 2, 1:NF + 1],
                            start=True, stop=True)
                    for m in (m0, m1):
                        if nch == 1:
                            nc.scalar.activation(dst[:, m, :], ips[m][:], Tanh,
                                                 scale=float(gains[kb]) / WI_SCALE)
                            pending_acc.append((m, dst, float(wmix[kb + 1])))
                        else:
                            if c == 0:
                                nc.vector.tensor_copy(inv_sb[:, m, :], ips[m][:])
                            elif c < nch - 1:
                                nc.vector.tensor_add(inv_sb[:, m, :], inv_sb[:, m, :], ips[m][:])
                            else:
                                nc.vector.tensor_add(inv_sb[:, m, :], inv_sb[:, m, :], ips[m][:])
                                nc.scalar.activation(dst[:, m, :], inv_sb[:, m, :], Tanh,
                                                     scale=float(gains[kb]) / WI_SCALE)
                                pending_acc.append((m, dst, float(wmix[kb + 1])))

        # final flush interleaved with the store crossbar transposes
        flush_acc(interleave=lambda m: nc.sync.dma_start_transpose(
            onat[:, :, m, :], accb[:, m, :]))
        if debug_taps:
            nc.sync.dma_start(taps["t_acc"][:], accb[:])
            nc.sync.dma_start(taps["t_onat"][:], onat[:])

        # ---- cast to fp32 (folding the uniform mixer weight) and store ----
        out_scale = float(wmix[0]) if w_equal else 1.0
        for b in range(BL):
            for jt in range(2):
                on32 = stream.tile([128, STEP], F32, tag="on32")
                if (b * 2 + jt) % 2 == 0:
                    nc.vector.tensor_scalar_mul(on32[:], onat[:, b * 2 + jt, :, :], out_scale)
                else:
                    nc.scalar.activation(on32[:], onat[:, b * 2 + jt, :, :], Copy,
                                         scale=out_scale)
                dma_eng = nc.sync if (b * 2 + jt) % 2 == 0 else nc.scalar
                dma_eng.dma_start(out_d[b, jt * 128:(jt + 1) * 128, :], on32[:])

    nc.compile()
    return nc


def _const_map(transfers, plans):
    cm = {}
    for kb in range(NBLK):
        mats = _host_matrices(transfers[kb].astype(np.float64), plans[kb])
        for c, md in enumerate(mats):
            cm[f"bf_{kb}_{c}"] = md["bf"]
            cm[f"wi_{kb}_{c}"] = md["winv"]
            cm[f"sg_{kb}_{c}"] = md["sign"]
            cm[f"tr_{kb}_{c}"] = md["tcrep"]
    return cm


def _in_maps(x, const_map):
    xr = np.ascontiguousarray(np.asarray(x, dtype=np.float32)).reshape(B, T)
    maps = []
    for core in range(NCORES):
        m = dict(const_map)
        m["x"] = np.ascontiguousarray(xr[core * BL:(core + 1) * BL].reshape(BL, NF, STEP))
        maps.append(m)
    return maps


def kernel(x, transfers, gains, mixer):
    transfers = np.asarray(transfers, dtype=np.float32)
    gains = np.asarray(gains, dtype=np.float64)
    mixer = np.asarray(mixer, dtype=np.float64)
    wm = np.exp(mixer - mixer.max())
    wm = wm / wm.sum()

    plans = [_plan_chunks(transfers[kb]) for kb in range(NBLK)]
    chunk_sizes = tuple(tuple(len(ch) for ch in pl) for pl in plans)
    key = (chunk_sizes, tuple(np.round(gains, 9)), tuple(np.round(wm, 9)))
    if key not in _CACHE:
        _CACHE[key] = _build(chunk_sizes, gains, wm)
    nc = _CACHE[key]

    in_maps = _in_maps(x, _const_map(transfers, plans))
    res = bass_utils.run_bass_kernel_spmd(nc, in_maps, core_ids=list(range(NCORES)))
    out = np.concatenate([res.results[i]["out"].reshape(BL, 1, T) for i in range(NCORES)], axis=0)
    return out.astype(np.float32)
